# revision 17
# baseline (speedup 1.0000x reference)
"""Trainium2 Bass kernel for nn_Entropy (histogram_binning): per-pixel Shannon
entropy of a 5x5-window KDE histogram over 256 intensity bins.

Math (validated in f32 vs reference):
  k(x,b) = sigmoid'(10(x-b)) = 0.25*(1 - tanh^2(5x-5b))   [exact identity]
  q[h,w,b] = 5x5 window sum of k;  S = sum_b q;  p = q/(S+EPS)
  out = -sum_b p*ln(p+EPS)
     computed as E = -r * sum_b q*ln(r*q+EPS),  r = 1/(S+EPS)
  S is computed analytically per pixel: s(x) = sum_o in {-1,0,1,2} of
  0.25*(1-tanh^2(5*frac(x)-5o)) masked at intensity range edges, then
  5x5-window-summed (tiny [96,96] work instead of a 256-bin reduction).

Layout per image: partitions = h (96), free = (w, b) with b inner (24576).
  - d' = 5x - 5b built by TensorE: rank-97 matmul with a shipped constant
    moving tensor CRHS[w', w*256+b] = 5*delta(w'==w), row 96 = -5b; the
    stationary is [5*x^T ; ones].
  - tanh on ScalarE evacuating PSUM; k = 0.25 - 0.25*t^2 on VectorE.
  - H-window: banded-matrix matmul (TensorE).  W-window: 5 shifted
    identity matmuls accumulating in PSUM (TensorE).
  - backend per w: L = ln(r*q + EPS) on ScalarE (per-partition scale AP),
    QL = sum_b -(q*L) via fused tensor_tensor_reduce on VectorE; E = r*QL.

Sharding: B*C = 24 images split 3-per-core across 8 cores; no collectives.
Self-contained: hardcodes shapes; builds/compiles the Bass module on first
call and reuses it.
"""

import sys

sys.path.insert(0, "/opt/trn_rl_repo")

import numpy as np

H = 96
W = 96
NB = 256
NIMG = 3          # images per core
NCORES = 8
FREE = W * NB     # 24576
EPS = 1e-10

_CACHE = {}


def _build_consts():
    # CRHSQ [5, 1024]: rows j=0..3 select w-offset j (value 1.0 over that
    # b-block); row 4 = -5*b tiled (the stationary carries 5*x^T and ones)
    crhs = np.zeros((5, 4 * NB), dtype=np.float32)
    for j in range(4):
        crhs[j, j * NB:(j + 1) * NB] = 1.0
    b = np.arange(NB, dtype=np.float32)
    crhs[4, :] = np.tile(-5.0 * b, 4)
    # banded H-window matrix [96, 96] (symmetric)
    hh = np.arange(H)
    band = (np.abs(hh[:, None] - hh[None, :]) <= 2).astype(np.float32)
    eye = np.eye(H, dtype=np.float32)
    return crhs, band, eye


def _emit_kernel(nc, tc, ctx, ins, outs):
    import concourse.bass as bass
    from concourse import mybir

    f32 = mybir.dt.float32
    AF = mybir.ActivationFunctionType
    OP = mybir.AluOpType

    x_d, xt_d, crhs_d, band_d, eye_d = ins
    (ent_d,) = outs

    consts = ctx.enter_context(tc.tile_pool(name="consts", bufs=1))
    big = ctx.enter_context(tc.tile_pool(name="big", bufs=1))
    sm = ctx.enter_context(tc.tile_pool(name="sm", bufs=1))
    chunks = ctx.enter_context(tc.tile_pool(name="chunks", bufs=3))
    psum = ctx.enter_context(tc.tile_pool(name="psum", bufs=4, space="PSUM"))

    # ---- load constants / inputs ----
    crhsq_sb = consts.tile([69, 4 * NB], f32)
    for k3 in range(3):
        nc.sync.dma_start(crhsq_sb[32 * k3:32 * k3 + 5, :], crhs_d[:])
    band_sb = consts.tile([H, H], f32)
    nc.sync.dma_start(band_sb[:], band_d[:])
    eye_sb = consts.tile([H, H], f32)
    nc.sync.dma_start(eye_sb[:], eye_d[:])

    xall = consts.tile([H, NIMG * W], f32)      # x[h, (i,w)]
    xtall = consts.tile([W, NIMG * H], f32)     # xT[w, (i,h)]
    for i in range(NIMG):
        nc.sync.dma_start(xall[:, i * W:(i + 1) * W], x_d[i])
        nc.sync.dma_start(xtall[:, i * H:(i + 1) * H], xt_d[i])

    # 24 stationary groups of 5 rows = [5*xT rows 4g..4g+3 ; ones], each in
    # its own tile (matmul lhsT base partition must be 0); the ones row is
    # DMA'd (compute engines can't write at partition offset 4)
    ones_sb = consts.tile([1, NIMG * H], f32)
    nc.vector.memset(ones_sb[:], 1.0)
    xt5_all = consts.tile([W, NIMG * H], f32)
    nc.vector.tensor_scalar(xt5_all[:], xtall[:], 5.0, None, op0=OP.mult)
    # 3 groups per tile at base partitions 0/32/64 (matmul lhsT constraint)
    xt5g = []
    for tg in range(8):
        gt = consts.tile([69, NIMG * H], f32, tag=f"xt5g{tg}")
        for k3 in range(3):
            g = tg * 3 + k3
            base = 32 * k3
            nc.sync.dma_start(gt[base:base + 4, :], xt5_all[4 * g:4 * g + 4, :])
            nc.sync.dma_start(gt[base + 4:base + 5, :], ones_sb[:])
            xt5g.append(gt[base:base + 5])

    # bias constant tiles (activation float bias needs a registered AP)
    bias_tiles = {}

    def bias_ap(val):
        if val not in bias_tiles:
            t = consts.tile([H, 1], f32, tag=f"bias{val}")
            nc.vector.memset(t[:], val)
            bias_tiles[val] = t
        return bias_tiles[val][:]

    # =====================  S path (tiny, [96, 288])  =====================
    NW = NIMG * W
    i32 = mybir.dt.int32
    ni = sm.tile([H, NW], i32)
    nc.vector.tensor_copy(ni[:], xall[:])     # f32 -> i32 (trunc or round)
    nf = sm.tile([H, NW], f32)
    nc.vector.tensor_copy(nf[:], ni[:])       # back to f32
    u = sm.tile([H, NW], f32)
    nc.vector.tensor_tensor(u[:], xall[:], nf[:], op=OP.subtract)
    # taps o in {-2..2}: v_o = tanh(5u - 5o); sq_o = v_o^2; masks from n:
    # bin n+o valid iff 0 <= n+o <= 255 (correct for either cast semantics)
    taps = (-2, -1, 0, 1, 2)
    sq = {}
    for o in taps:
        v = sm.tile([H, NW], f32, tag=f"v{o}")
        nc.scalar.activation(v[:], u[:], AF.Tanh, bias=bias_ap(-5.0 * o), scale=5.0)
        s2 = sm.tile([H, NW], f32, tag=f"sq{o}")
        nc.scalar.activation(s2[:], v[:], AF.Square)
        sq[o] = s2
    masks = {}
    for o in taps:
        if o == 0:
            continue
        m = sm.tile([H, NW], f32, tag=f"m{o}")
        if o < 0:
            nc.vector.tensor_scalar(m[:], nf[:], float(-o), None, op0=OP.is_ge)
        else:
            nc.vector.tensor_scalar(m[:], nf[:], float(255 - o), None, op0=OP.is_le)
        masks[o] = m
    # cnt = 1 + sum_o masks ; ssum = sq0 + sum_o m_o*sq_o
    cnt = sm.tile([H, NW], f32)
    nc.vector.tensor_tensor(cnt[:], masks[-2][:], masks[-1][:], op=OP.add)
    nc.vector.tensor_tensor(cnt[:], cnt[:], masks[1][:], op=OP.add)
    nc.vector.tensor_tensor(cnt[:], cnt[:], masks[2][:], op=OP.add)
    nc.vector.tensor_scalar(cnt[:], cnt[:], 1.0, None, op0=OP.add)
    ssum = sm.tile([H, NW], f32)
    nc.vector.tensor_copy(ssum[:], sq[0][:])
    for o in (-2, -1, 1, 2):
        t_m = sm.tile([H, NW], f32, tag=f"tm{o}")
        nc.vector.tensor_tensor(t_m[:], masks[o][:], sq[o][:], op=OP.mult)
        nc.vector.tensor_tensor(ssum[:], ssum[:], t_m[:], op=OP.add)
    # s_pix = 0.25*cnt - 0.25*ssum
    spix = sm.tile([H, NW], f32)
    nc.vector.tensor_tensor(spix[:], cnt[:], ssum[:], op=OP.subtract)
    nc.vector.tensor_scalar(spix[:], spix[:], 0.25, None, op0=OP.mult)
    # H window via band matmul
    ps_s = psum.tile([H, 1024], f32, tag="ps")
    nc.tensor.matmul(ps_s[:, 0:NW], band_sb[:], spix[:], start=True, stop=True)
    sh = sm.tile([H, NW], f32)
    nc.scalar.copy(sh[:], ps_s[:, 0:NW])
    # W window via padded shifted adds: shp [H, NIMG, 100] zero-padded
    shp = sm.tile([H, NIMG, W + 4], f32)
    nc.vector.memset(shp[:], 0.0)
    for i in range(NIMG):
        nc.vector.tensor_copy(shp[:, i, 2:2 + W], sh[:, i * W:(i + 1) * W])
    swin = sm.tile([H, NIMG, W], f32)
    nc.vector.tensor_tensor(swin[:], shp[:, :, 0:W], shp[:, :, 1:1 + W], op=OP.add)
    for j in (2, 3, 4):
        nc.vector.tensor_tensor(swin[:], swin[:], shp[:, :, j:j + W], op=OP.add)
    # r = 1/(S + EPS)
    rtile = sm.tile([H, NW], f32)
    sw_flat = swin[:].rearrange("p a b -> p (a b)")
    nc.vector.tensor_scalar(rtile[:], sw_flat, EPS, None, op0=OP.add)
    nc.vector.reciprocal(rtile[:], rtile[:])

    # =====================  main dense path, per image  =====================
    QL = sm.tile([H, NW], f32)

    for i in range(NIMG):
        qh = big.tile([H, FREE], f32, tag="qh")

        # ---- front end: d' -> tanh -> t^2 -> k -> H-band -> qh ----
        NDT = 24  # d-psum tiles of [96, 1024] per image
        for c in range(NDT):
            pd = psum.tile([H, 1024], f32, tag="ps")
            for piece in range(2):
                p_abs = c * 2 + piece          # 512-col piece index, 0..47
                g = p_abs // 2                 # w-quad group
                half = p_abs % 2
                base = 32 * (g % 3)
                nc.tensor.matmul(
                    pd[:, piece * 512:(piece + 1) * 512],
                    xt5g[g][:, i * H:(i + 1) * H],
                    crhsq_sb[base:base + 5, half * 512:(half + 1) * 512],
                    start=True,
                    stop=True,
                )
            tt = chunks.tile([H, 1024], f32, tag="t")
            nc.scalar.activation(tt[:], pd[:], AF.Tanh)
            kk = chunks.tile([H, 1024], f32, tag="k")
            nc.vector.tensor_tensor(kk[:], tt[:], tt[:], op=OP.mult)
            nc.vector.tensor_scalar(kk[:], kk[:], -0.25, 0.25, op0=OP.mult, op1=OP.add)
            ph = psum.tile([H, 1024], f32, tag="ps")
            for piece in range(2):
                nc.tensor.matmul(
                    ph[:, piece * 512:(piece + 1) * 512],
                    band_sb[:],
                    kk[:, piece * 512:(piece + 1) * 512],
                    start=True,
                    stop=True,
                )
            nc.scalar.copy(qh[:, c * 1024:(c + 1) * 1024], ph[:])

        # ---- W window + backend, per 4-w tile ----
        for wc in range(W // 4):
            w0 = 4 * wc
            pw = psum.tile([H, 1024], f32, tag="ps")
            for j in range(4):
                w = w0 + j
                valid = [s for s in (-2, -1, 0, 1, 2) if 0 <= w + s <= W - 1]
                for si, s in enumerate(valid):
                    nc.tensor.matmul(
                        pw[:, j * 256:(j + 1) * 256],
                        eye_sb[:],
                        qh[:, (w + s) * NB:(w + s + 1) * NB],
                        start=(si == 0),
                        stop=(si == len(valid) - 1),
                    )
            ltile = chunks.tile([H, 1024], f32, tag="L")
            for j in range(4):
                w = w0 + j
                rcol = rtile[:, i * W + w:i * W + w + 1]
                nc.scalar.activation(
                    ltile[:, j * 256:(j + 1) * 256],
                    pw[:, j * 256:(j + 1) * 256],
                    AF.Ln,
                    bias=bias_ap(EPS),
                    scale=rcol,
                )
            etile = chunks.tile([H, 1024], f32, tag="e")
            nc.vector.tensor_tensor(etile[:], pw[:], ltile[:], op=OP.mult)
            nc.vector.tensor_reduce(
                QL[:, i * W + w0:i * W + w0 + 4],
                etile[:].rearrange("p (a b) -> p a b", b=NB),
                axis=mybir.AxisListType.X,
                op=OP.add,
            )

    # E = r * QL ; write out
    ent = sm.tile([H, NW], f32)
    nc.vector.tensor_tensor(ent[:], rtile[:], QL[:], op=OP.mult)
    nc.vector.tensor_scalar(ent[:], ent[:], -1.0, None, op0=OP.mult)
    for i in range(NIMG):
        nc.sync.dma_start(ent_d[i], ent[:, i * W:(i + 1) * W])


def _get_compiled():
    if "nc" in _CACHE:
        return _CACHE["nc"]
    from contextlib import ExitStack

    import concourse.bass as bass
    import concourse.tile as tile
    from concourse import bacc, mybir

    f32 = mybir.dt.float32
    nc = bacc.Bacc("TRN2", target_bir_lowering=False, debug=False)
    x_d = nc.dram_tensor("x_sh", [NIMG, H, W], f32, kind="ExternalInput").ap()
    xt_d = nc.dram_tensor("xt_sh", [NIMG, W, H], f32, kind="ExternalInput").ap()
    crhs_d = nc.dram_tensor("crhs", [5, 4 * NB], f32, kind="ExternalInput").ap()
    band_d = nc.dram_tensor("bandh", [H, H], f32, kind="ExternalInput").ap()
    eye_d = nc.dram_tensor("i96", [H, H], f32, kind="ExternalInput").ap()
    ent_d = nc.dram_tensor("ent", [NIMG, H, W], f32, kind="ExternalOutput").ap()

    with tile.TileContext(nc) as tc:
        with ExitStack() as ctx:
            _emit_kernel(nc, tc, ctx, (x_d, xt_d, crhs_d, band_d, eye_d), (ent_d,))
    nc.compile()
    _CACHE["nc"] = nc
    return nc


def make_in_maps(x):
    """x: full [8, 3, 96, 96] -> list of 8 per-core input dicts."""
    x = np.ascontiguousarray(np.asarray(x, dtype=np.float32))
    imgs = x.reshape(NCORES * NIMG, H, W)
    crhs, band, eye = _build_consts()
    in_maps = []
    for c in range(NCORES):
        sh = np.ascontiguousarray(imgs[c * NIMG:(c + 1) * NIMG])
        in_maps.append(
            {
                "x_sh": sh,
                "xt_sh": np.ascontiguousarray(sh.transpose(0, 2, 1)),
                "crhs": crhs,
                "bandh": band,
                "i96": eye,
            }
        )
    return in_maps


def kernel(x):
    """Full inputs in, full outputs out. x: [8, 3, 96, 96] f32."""
    from concourse.bass_utils import run_bass_kernel_spmd

    nc = _get_compiled()
    in_maps = make_in_maps(x)
    res = run_bass_kernel_spmd(nc, in_maps, list(range(NCORES)))
    out = np.stack([res.results[c]["ent"] for c in range(NCORES)])  # [8, 3, H, W]
    return out.reshape(8, 3, H, W).astype(np.float32)


# revision 20
# speedup vs baseline: 1.1535x; 1.1535x over previous
"""Trainium2 Bass kernel for nn_Entropy (histogram_binning): per-pixel Shannon
entropy of a 5x5-window KDE histogram over 256 intensity bins.

Math (validated in f32 vs reference):
  k(x,b) = sigmoid'(10(x-b)) = 0.25*(1 - tanh^2(5x-5b))   [exact identity]
  q[h,w,b] = 5x5 window sum of k;  S = sum_b q;  p = q/(S+EPS)
  out = -sum_b p*ln(p+EPS)
     computed as E = -r * sum_b q*ln(r*q+EPS),  r = 1/(S+EPS)
  S is computed analytically per pixel: s(x) = sum_o in {-1,0,1,2} of
  0.25*(1-tanh^2(5*frac(x)-5o)) masked at intensity range edges, then
  5x5-window-summed (tiny [96,96] work instead of a 256-bin reduction).

Layout per image: partitions = h (96), free = (w, b) with b inner (24576).
  - d' = 5x - 5b built by TensorE: rank-97 matmul with a shipped constant
    moving tensor CRHS[w', w*256+b] = 5*delta(w'==w), row 96 = -5b; the
    stationary is [5*x^T ; ones].
  - tanh on ScalarE evacuating PSUM; k = 0.25 - 0.25*t^2 on VectorE.
  - H-window: banded-matrix matmul (TensorE).  W-window: 5 shifted
    identity matmuls accumulating in PSUM (TensorE).
  - backend per w: L = ln(r*q + EPS) on ScalarE (per-partition scale AP),
    QL = sum_b -(q*L) via fused tensor_tensor_reduce on VectorE; E = r*QL.

Sharding: B*C = 24 images split 3-per-core across 8 cores; no collectives.
Self-contained: hardcodes shapes; builds/compiles the Bass module on first
call and reuses it.
"""

import sys

sys.path.insert(0, "/opt/trn_rl_repo")

import numpy as np

H = 96
W = 96
NB = 256
NIMG = 3          # images per core
NCORES = 8
FREE = W * NB     # 24576
EPS = 1e-10

_CACHE = {}


def _build_consts():
    # CRHSQ [5, 1024]: rows j=0..3 select w-offset j (value 1.0 over that
    # b-block); row 4 = -5*b tiled (the stationary carries 5*x^T and ones)
    crhs = np.zeros((5, 4 * NB), dtype=np.float32)
    for j in range(4):
        crhs[j, j * NB:(j + 1) * NB] = 1.0
    b = np.arange(NB, dtype=np.float32)
    crhs[4, :] = np.tile(-5.0 * b, 4)
    # banded H-window matrix [96, 96] (symmetric)
    hh = np.arange(H)
    band = (np.abs(hh[:, None] - hh[None, :]) <= 2).astype(np.float32)
    eye = np.eye(H, dtype=np.float32)
    return crhs, band, eye


def _emit_kernel(nc, tc, ctx, ins, outs):
    import concourse.bass as bass
    from concourse import mybir

    f32 = mybir.dt.float32
    AF = mybir.ActivationFunctionType
    OP = mybir.AluOpType

    x_d, xt_d, crhs_d, band_d, eye_d = ins
    (ent_d,) = outs

    consts = ctx.enter_context(tc.tile_pool(name="consts", bufs=1))
    big = ctx.enter_context(tc.tile_pool(name="big", bufs=1))
    sm = ctx.enter_context(tc.tile_pool(name="sm", bufs=1))
    chunks = ctx.enter_context(tc.tile_pool(name="chunks", bufs=3))
    chunks2 = ctx.enter_context(tc.tile_pool(name="chunks2", bufs=2))
    psum = ctx.enter_context(tc.tile_pool(name="psum", bufs=4, space="PSUM"))

    # ---- load constants / inputs ----
    crhsq_sb = consts.tile([69, 4 * NB], f32)
    for k3 in range(3):
        nc.sync.dma_start(crhsq_sb[32 * k3:32 * k3 + 5, :], crhs_d[:])
    band_sb = consts.tile([H, H], f32)
    nc.sync.dma_start(band_sb[:], band_d[:])

    xall = consts.tile([H, NIMG * W], f32)      # x[h, (i,w)]
    xtall = consts.tile([W, NIMG * H], f32)     # xT[w, (i,h)]
    for i in range(NIMG):
        nc.sync.dma_start(xall[:, i * W:(i + 1) * W], x_d[i])
        nc.sync.dma_start(xtall[:, i * H:(i + 1) * H], xt_d[i])

    # 24 stationary groups of 5 rows = [5*xT rows 4g..4g+3 ; ones], each in
    # its own tile (matmul lhsT base partition must be 0); the ones row is
    # DMA'd (compute engines can't write at partition offset 4)
    ones_sb = consts.tile([1, NIMG * H], f32)
    nc.vector.memset(ones_sb[:], 1.0)
    xt5_all = consts.tile([W, NIMG * H], f32)
    nc.vector.tensor_scalar(xt5_all[:], xtall[:], 5.0, None, op0=OP.mult)
    # 3 groups per tile at base partitions 0/32/64 (matmul lhsT constraint)
    xt5g = []
    for tg in range(8):
        gt = consts.tile([69, NIMG * H], f32, tag=f"xt5g{tg}")
        for k3 in range(3):
            g = tg * 3 + k3
            base = 32 * k3
            nc.sync.dma_start(gt[base:base + 4, :], xt5_all[4 * g:4 * g + 4, :])
            nc.sync.dma_start(gt[base + 4:base + 5, :], ones_sb[:])
            xt5g.append(gt[base:base + 5])

    # bias constant tiles (activation float bias needs a registered AP)
    bias_tiles = {}

    def bias_ap(val):
        if val not in bias_tiles:
            t = consts.tile([H, 1], f32, tag=f"bias{val}")
            nc.vector.memset(t[:], val)
            bias_tiles[val] = t
        return bias_tiles[val][:]

    # =====================  S path (tiny, [96, 288])  =====================
    NW = NIMG * W
    i32 = mybir.dt.int32
    ni = sm.tile([H, NW], i32)
    nc.vector.tensor_copy(ni[:], xall[:])     # f32 -> i32 (trunc or round)
    nf = sm.tile([H, NW], f32)
    nc.vector.tensor_copy(nf[:], ni[:])       # back to f32
    u = sm.tile([H, NW], f32)
    nc.vector.tensor_tensor(u[:], xall[:], nf[:], op=OP.subtract)
    # taps o in {-2..2}: v_o = tanh(5u - 5o); sq_o = v_o^2; masks from n:
    # bin n+o valid iff 0 <= n+o <= 255 (correct for either cast semantics)
    taps = (-2, -1, 0, 1, 2)
    sq = {}
    for o in taps:
        v = sm.tile([H, NW], f32, tag=f"v{o}")
        nc.scalar.activation(v[:], u[:], AF.Tanh, bias=bias_ap(-5.0 * o), scale=5.0)
        s2 = sm.tile([H, NW], f32, tag=f"sq{o}")
        nc.scalar.activation(s2[:], v[:], AF.Square)
        sq[o] = s2
    masks = {}
    for o in taps:
        if o == 0:
            continue
        m = sm.tile([H, NW], f32, tag=f"m{o}")
        if o < 0:
            nc.vector.tensor_scalar(m[:], nf[:], float(-o), None, op0=OP.is_ge)
        else:
            nc.vector.tensor_scalar(m[:], nf[:], float(255 - o), None, op0=OP.is_le)
        masks[o] = m
    # cnt = 1 + sum_o masks ; ssum = sq0 + sum_o m_o*sq_o
    cnt = sm.tile([H, NW], f32)
    nc.vector.tensor_tensor(cnt[:], masks[-2][:], masks[-1][:], op=OP.add)
    nc.vector.tensor_tensor(cnt[:], cnt[:], masks[1][:], op=OP.add)
    nc.vector.tensor_tensor(cnt[:], cnt[:], masks[2][:], op=OP.add)
    nc.vector.tensor_scalar(cnt[:], cnt[:], 1.0, None, op0=OP.add)
    ssum = sm.tile([H, NW], f32)
    nc.vector.tensor_copy(ssum[:], sq[0][:])
    for o in (-2, -1, 1, 2):
        t_m = sm.tile([H, NW], f32, tag=f"tm{o}")
        nc.vector.tensor_tensor(t_m[:], masks[o][:], sq[o][:], op=OP.mult)
        nc.vector.tensor_tensor(ssum[:], ssum[:], t_m[:], op=OP.add)
    # s_pix = 0.25*cnt - 0.25*ssum
    spix = sm.tile([H, NW], f32)
    nc.vector.tensor_tensor(spix[:], cnt[:], ssum[:], op=OP.subtract)
    nc.vector.tensor_scalar(spix[:], spix[:], 0.25, None, op0=OP.mult)
    # H window via band matmul
    ps_s = psum.tile([H, 1024], f32, tag="ps")
    nc.tensor.matmul(ps_s[:, 0:NW], band_sb[:], spix[:], start=True, stop=True)
    sh = sm.tile([H, NW], f32)
    nc.scalar.copy(sh[:], ps_s[:, 0:NW])
    # W window via padded shifted adds: shp [H, NIMG, 100] zero-padded
    shp = sm.tile([H, NIMG, W + 4], f32)
    nc.vector.memset(shp[:], 0.0)
    for i in range(NIMG):
        nc.vector.tensor_copy(shp[:, i, 2:2 + W], sh[:, i * W:(i + 1) * W])
    swin = sm.tile([H, NIMG, W], f32)
    nc.vector.tensor_tensor(swin[:], shp[:, :, 0:W], shp[:, :, 1:1 + W], op=OP.add)
    for j in (2, 3, 4):
        nc.vector.tensor_tensor(swin[:], swin[:], shp[:, :, j:j + W], op=OP.add)
    # r = 1/(S + EPS)
    rtile = sm.tile([H, NW], f32)
    sw_flat = swin[:].rearrange("p a b -> p (a b)")
    nc.vector.tensor_scalar(rtile[:], sw_flat, EPS, None, op0=OP.add)
    nc.vector.reciprocal(rtile[:], rtile[:])

    # =====================  main dense path, per image  =====================
    QL = sm.tile([H, NW], f32)

    for i in range(NIMG):
        qh = big.tile([H, FREE], f32, tag="qh")

        # ---- front end: d' -> tanh -> t^2 -> k -> H-band -> qh ----
        NDT = 24  # d-psum tiles of [96, 1024] per image
        for c in range(NDT):
            pd = psum.tile([H, 1024], f32, tag="ps")
            for piece in range(2):
                p_abs = c * 2 + piece          # 512-col piece index, 0..47
                g = p_abs // 2                 # w-quad group
                half = p_abs % 2
                base = 32 * (g % 3)
                nc.tensor.matmul(
                    pd[:, piece * 512:(piece + 1) * 512],
                    xt5g[g][:, i * H:(i + 1) * H],
                    crhsq_sb[base:base + 5, half * 512:(half + 1) * 512],
                    start=True,
                    stop=True,
                )
            tt = chunks.tile([H, 1024], f32, tag="t")
            nc.scalar.activation(tt[:], pd[:], AF.Tanh)
            kk = chunks.tile([H, 1024], f32, tag="k")
            nc.vector.tensor_tensor(kk[:], tt[:], tt[:], op=OP.mult)
            nc.vector.tensor_scalar(kk[:], kk[:], -0.25, 0.25, op0=OP.mult, op1=OP.add)
            ph = psum.tile([H, 1024], f32, tag="ps")
            for piece in range(2):
                nc.tensor.matmul(
                    ph[:, piece * 512:(piece + 1) * 512],
                    band_sb[:],
                    kk[:, piece * 512:(piece + 1) * 512],
                    start=True,
                    stop=True,
                )
            if c % 2 == 0:
                nc.scalar.copy(qh[:, c * 1024:(c + 1) * 1024], ph[:])
            else:
                nc.vector.tensor_copy(qh[:, c * 1024:(c + 1) * 1024], ph[:])

        # ---- W window: in-place prefix scan along w per bin, then shifted
        # differences: q[w] = P[min(w+2,95)] - (P[w-3] if w>=3 else 0) ----
        qh3 = qh[:].rearrange("p (w b) -> p w b", b=NB)
        for b in range(NB):
            ap = qh[:, b::NB]
            nc.vector.tensor_tensor_scan(
                ap, ap, ap, 0.0, op0=OP.add, op1=OP.bypass
            )

        for wc in range(W // 4):
            w0 = 4 * wc
            qt = chunks2.tile([H, 4, NB], f32, tag="q")
            if wc == 0:
                # w=0,1,2: q = P[w+2]; w=3: q = P[5] - P[0]
                nc.vector.tensor_copy(qt[:, 0:3, :], qh3[:, 2:5, :])
                nc.vector.tensor_tensor(
                    qt[:, 3:4, :], qh3[:, 5:6, :], qh3[:, 0:1, :], op=OP.subtract
                )
            elif wc == W // 4 - 1:
                # w=92,93 interior; w=94,95: q = P[95] - P[w-3]
                nc.vector.tensor_tensor(
                    qt[:, 0:2, :], qh3[:, 94:96, :], qh3[:, 89:91, :],
                    op=OP.subtract,
                )
                nc.vector.tensor_tensor(
                    qt[:, 2:3, :], qh3[:, 95:96, :], qh3[:, 91:92, :],
                    op=OP.subtract,
                )
                nc.vector.tensor_tensor(
                    qt[:, 3:4, :], qh3[:, 95:96, :], qh3[:, 92:93, :],
                    op=OP.subtract,
                )
            else:
                nc.vector.tensor_tensor(
                    qt[:], qh3[:, w0 + 2:w0 + 6, :], qh3[:, w0 - 3:w0 + 1, :],
                    op=OP.subtract,
                )
            ltile = chunks.tile([H, 1024], f32, tag="L")
            for j in range(4):
                w = w0 + j
                rcol = rtile[:, i * W + w:i * W + w + 1]
                nc.scalar.activation(
                    ltile[:, j * 256:(j + 1) * 256],
                    qt[:, j, :],
                    AF.Ln,
                    bias=bias_ap(EPS),
                    scale=rcol,
                )
            etile = chunks2.tile([H, 1024], f32, tag="e")
            nc.vector.tensor_tensor(
                etile[:].rearrange("p (a b) -> p a b", b=NB), qt[:], 
                ltile[:].rearrange("p (a b) -> p a b", b=NB), op=OP.mult
            )
            nc.vector.tensor_reduce(
                QL[:, i * W + w0:i * W + w0 + 4],
                etile[:].rearrange("p (a b) -> p a b", b=NB),
                axis=mybir.AxisListType.X,
                op=OP.add,
            )

    # E = r * QL ; write out
    ent = sm.tile([H, NW], f32)
    nc.vector.tensor_tensor(ent[:], rtile[:], QL[:], op=OP.mult)
    nc.vector.tensor_scalar(ent[:], ent[:], -1.0, None, op0=OP.mult)
    for i in range(NIMG):
        nc.sync.dma_start(ent_d[i], ent[:, i * W:(i + 1) * W])


def _get_compiled():
    if "nc" in _CACHE:
        return _CACHE["nc"]
    from contextlib import ExitStack

    import concourse.bass as bass
    import concourse.tile as tile
    from concourse import bacc, mybir

    f32 = mybir.dt.float32
    nc = bacc.Bacc("TRN2", target_bir_lowering=False, debug=False)
    x_d = nc.dram_tensor("x_sh", [NIMG, H, W], f32, kind="ExternalInput").ap()
    xt_d = nc.dram_tensor("xt_sh", [NIMG, W, H], f32, kind="ExternalInput").ap()
    crhs_d = nc.dram_tensor("crhs", [5, 4 * NB], f32, kind="ExternalInput").ap()
    band_d = nc.dram_tensor("bandh", [H, H], f32, kind="ExternalInput").ap()
    eye_d = nc.dram_tensor("i96", [H, H], f32, kind="ExternalInput").ap()
    ent_d = nc.dram_tensor("ent", [NIMG, H, W], f32, kind="ExternalOutput").ap()

    with tile.TileContext(nc) as tc:
        with ExitStack() as ctx:
            _emit_kernel(nc, tc, ctx, (x_d, xt_d, crhs_d, band_d, eye_d), (ent_d,))
    nc.compile()
    _CACHE["nc"] = nc
    return nc


def make_in_maps(x):
    """x: full [8, 3, 96, 96] -> list of 8 per-core input dicts."""
    x = np.ascontiguousarray(np.asarray(x, dtype=np.float32))
    imgs = x.reshape(NCORES * NIMG, H, W)
    crhs, band, eye = _build_consts()
    in_maps = []
    for c in range(NCORES):
        sh = np.ascontiguousarray(imgs[c * NIMG:(c + 1) * NIMG])
        in_maps.append(
            {
                "x_sh": sh,
                "xt_sh": np.ascontiguousarray(sh.transpose(0, 2, 1)),
                "crhs": crhs,
                "bandh": band,
                "i96": eye,
            }
        )
    return in_maps


def kernel(x):
    """Full inputs in, full outputs out. x: [8, 3, 96, 96] f32."""
    from concourse.bass_utils import run_bass_kernel_spmd

    nc = _get_compiled()
    in_maps = make_in_maps(x)
    res = run_bass_kernel_spmd(nc, in_maps, list(range(NCORES)))
    out = np.stack([res.results[c]["ent"] for c in range(NCORES)])  # [8, 3, H, W]
    return out.reshape(8, 3, H, W).astype(np.float32)


# revision 22
# speedup vs baseline: 1.1770x; 1.0204x over previous
"""Trainium2 Bass kernel for nn_Entropy (histogram_binning): per-pixel Shannon
entropy of a 5x5-window KDE histogram over 256 intensity bins.

Math (validated in f32 vs reference):
  k(x,b) = sigmoid'(10(x-b)) = 0.25*(1 - tanh^2(5x-5b))   [exact identity]
  q[h,w,b] = 5x5 window sum of k;  S = sum_b q;  p = q/(S+EPS)
  out = -sum_b p*ln(p+EPS)
     computed as E = -r * sum_b q*ln(r*q+EPS),  r = 1/(S+EPS)
  S is computed analytically per pixel: s(x) = sum_o in {-1,0,1,2} of
  0.25*(1-tanh^2(5*frac(x)-5o)) masked at intensity range edges, then
  5x5-window-summed (tiny [96,96] work instead of a 256-bin reduction).

Layout per image: partitions = h (96), free = (w, b) with b inner (24576).
  - d' = 5x - 5b built by TensorE: rank-97 matmul with a shipped constant
    moving tensor CRHS[w', w*256+b] = 5*delta(w'==w), row 96 = -5b; the
    stationary is [5*x^T ; ones].
  - tanh on ScalarE evacuating PSUM; k = 0.25 - 0.25*t^2 on VectorE.
  - H-window: banded-matrix matmul (TensorE).  W-window: 5 shifted
    identity matmuls accumulating in PSUM (TensorE).
  - backend per w: L = ln(r*q + EPS) on ScalarE (per-partition scale AP),
    QL = sum_b -(q*L) via fused tensor_tensor_reduce on VectorE; E = r*QL.

Sharding: B*C = 24 images split 3-per-core across 8 cores; no collectives.
Self-contained: hardcodes shapes; builds/compiles the Bass module on first
call and reuses it.
"""

import sys

sys.path.insert(0, "/opt/trn_rl_repo")

import numpy as np

H = 96
W = 96
NB = 256
NIMG = 3          # images per core
NCORES = 8
FREE = W * NB     # 24576
EPS = 1e-10

_CACHE = {}


def _build_consts():
    # CRHSQ [5, 1024]: rows j=0..3 select w-offset j (value 1.0 over that
    # b-block); row 4 = -5*b tiled (the stationary carries 5*x^T and ones)
    crhs = np.zeros((5, 4 * NB), dtype=np.float32)
    for j in range(4):
        crhs[j, j * NB:(j + 1) * NB] = 1.0
    b = np.arange(NB, dtype=np.float32)
    crhs[4, :] = np.tile(-5.0 * b, 4)
    # banded H-window matrix [96, 96] (symmetric)
    hh = np.arange(H)
    band = (np.abs(hh[:, None] - hh[None, :]) <= 2).astype(np.float32)
    eye = np.eye(H, dtype=np.float32)
    return crhs, band, eye


def _emit_kernel(nc, tc, ctx, ins, outs):
    import concourse.bass as bass
    from concourse import mybir

    f32 = mybir.dt.float32
    AF = mybir.ActivationFunctionType
    OP = mybir.AluOpType

    x_d, xt_d, crhs_d, band_d, eye_d = ins
    (ent_d,) = outs

    consts = ctx.enter_context(tc.tile_pool(name="consts", bufs=1))
    big = ctx.enter_context(tc.tile_pool(name="big", bufs=1))
    sm = ctx.enter_context(tc.tile_pool(name="sm", bufs=1))
    chunks = ctx.enter_context(tc.tile_pool(name="chunks", bufs=3))
    chunks2 = ctx.enter_context(tc.tile_pool(name="chunks2", bufs=2))
    psum = ctx.enter_context(tc.tile_pool(name="psum", bufs=4, space="PSUM"))

    # ---- load constants / inputs ----
    crhsq_sb = consts.tile([69, 4 * NB], f32)
    for k3 in range(3):
        nc.sync.dma_start(crhsq_sb[32 * k3:32 * k3 + 5, :], crhs_d[:])
    band_sb = consts.tile([H, H], f32)
    nc.sync.dma_start(band_sb[:], band_d[:])

    xall = consts.tile([H, NIMG * W], f32)      # x[h, (i,w)]
    xtall = consts.tile([W, NIMG * H], f32)     # xT[w, (i,h)]
    for i in range(NIMG):
        nc.sync.dma_start(xall[:, i * W:(i + 1) * W], x_d[i])
        nc.sync.dma_start(xtall[:, i * H:(i + 1) * H], xt_d[i])

    # 24 stationary groups of 5 rows = [5*xT rows 4g..4g+3 ; ones], each in
    # its own tile (matmul lhsT base partition must be 0); the ones row is
    # DMA'd (compute engines can't write at partition offset 4)
    ones_sb = consts.tile([1, NIMG * H], f32)
    nc.vector.memset(ones_sb[:], 1.0)
    xt5_all = consts.tile([W, NIMG * H], f32)
    nc.vector.tensor_scalar(xt5_all[:], xtall[:], 5.0, None, op0=OP.mult)
    # 3 groups per tile at base partitions 0/32/64 (matmul lhsT constraint)
    xt5g = []
    for tg in range(8):
        gt = consts.tile([69, NIMG * H], f32, tag=f"xt5g{tg}")
        for k3 in range(3):
            g = tg * 3 + k3
            base = 32 * k3
            nc.sync.dma_start(gt[base:base + 4, :], xt5_all[4 * g:4 * g + 4, :])
            nc.sync.dma_start(gt[base + 4:base + 5, :], ones_sb[:])
            xt5g.append(gt[base:base + 5])

    # bias constant tiles (activation float bias needs a registered AP)
    bias_tiles = {}

    def bias_ap(val):
        if val not in bias_tiles:
            t = consts.tile([H, 1], f32, tag=f"bias{val}")
            nc.vector.memset(t[:], val)
            bias_tiles[val] = t
        return bias_tiles[val][:]

    # =====================  S path (tiny, [96, 288])  =====================
    NW = NIMG * W
    i32 = mybir.dt.int32
    ni = sm.tile([H, NW], i32)
    nc.vector.tensor_copy(ni[:], xall[:])     # f32 -> i32 (trunc or round)
    nf = sm.tile([H, NW], f32)
    nc.vector.tensor_copy(nf[:], ni[:])       # back to f32
    u = sm.tile([H, NW], f32)
    nc.vector.tensor_tensor(u[:], xall[:], nf[:], op=OP.subtract)
    # taps o in {-2..2}: v_o = tanh(5u - 5o); sq_o = v_o^2; masks from n:
    # bin n+o valid iff 0 <= n+o <= 255 (correct for either cast semantics)
    taps = (-2, -1, 0, 1, 2)
    sq = {}
    for o in taps:
        v = sm.tile([H, NW], f32, tag=f"v{o}")
        nc.scalar.activation(v[:], u[:], AF.Tanh, bias=bias_ap(-5.0 * o), scale=5.0)
        s2 = sm.tile([H, NW], f32, tag=f"sq{o}")
        nc.scalar.activation(s2[:], v[:], AF.Square)
        sq[o] = s2
    masks = {}
    for o in taps:
        if o == 0:
            continue
        m = sm.tile([H, NW], f32, tag=f"m{o}")
        if o < 0:
            nc.vector.tensor_scalar(m[:], nf[:], float(-o), None, op0=OP.is_ge)
        else:
            nc.vector.tensor_scalar(m[:], nf[:], float(255 - o), None, op0=OP.is_le)
        masks[o] = m
    # cnt = 1 + sum_o masks ; ssum = sq0 + sum_o m_o*sq_o
    cnt = sm.tile([H, NW], f32)
    nc.vector.tensor_tensor(cnt[:], masks[-2][:], masks[-1][:], op=OP.add)
    nc.vector.tensor_tensor(cnt[:], cnt[:], masks[1][:], op=OP.add)
    nc.vector.tensor_tensor(cnt[:], cnt[:], masks[2][:], op=OP.add)
    nc.vector.tensor_scalar(cnt[:], cnt[:], 1.0, None, op0=OP.add)
    ssum = sm.tile([H, NW], f32)
    nc.vector.tensor_copy(ssum[:], sq[0][:])
    for o in (-2, -1, 1, 2):
        t_m = sm.tile([H, NW], f32, tag=f"tm{o}")
        nc.vector.tensor_tensor(t_m[:], masks[o][:], sq[o][:], op=OP.mult)
        nc.vector.tensor_tensor(ssum[:], ssum[:], t_m[:], op=OP.add)
    # s_pix = 0.25*cnt - 0.25*ssum
    spix = sm.tile([H, NW], f32)
    nc.vector.tensor_tensor(spix[:], cnt[:], ssum[:], op=OP.subtract)
    nc.vector.tensor_scalar(spix[:], spix[:], 0.25, None, op0=OP.mult)
    # H window via band matmul
    ps_s = psum.tile([H, 1024], f32, tag="ps")
    nc.tensor.matmul(ps_s[:, 0:NW], band_sb[:], spix[:], start=True, stop=True)
    sh = sm.tile([H, NW], f32)
    nc.scalar.copy(sh[:], ps_s[:, 0:NW])
    # W window via padded shifted adds: shp [H, NIMG, 100] zero-padded
    shp = sm.tile([H, NIMG, W + 4], f32)
    nc.vector.memset(shp[:], 0.0)
    for i in range(NIMG):
        nc.vector.tensor_copy(shp[:, i, 2:2 + W], sh[:, i * W:(i + 1) * W])
    swin = sm.tile([H, NIMG, W], f32)
    nc.vector.tensor_tensor(swin[:], shp[:, :, 0:W], shp[:, :, 1:1 + W], op=OP.add)
    for j in (2, 3, 4):
        nc.vector.tensor_tensor(swin[:], swin[:], shp[:, :, j:j + W], op=OP.add)
    # r = 1/(S + EPS)
    rtile = sm.tile([H, NW], f32)
    sw_flat = swin[:].rearrange("p a b -> p (a b)")
    nc.vector.tensor_scalar(rtile[:], sw_flat, EPS, None, op0=OP.add)
    nc.vector.reciprocal(rtile[:], rtile[:])

    # =====================  main dense path, per image  =====================
    QL = sm.tile([H, NW], f32)

    ZB = 99  # per-bin block: 3 zero pads + 96 w columns
    for i in range(NIMG):
        qh = big.tile([H, NB * ZB + 8], f32, tag="qh")
        qh3 = qh[:, 0:NB * ZB].rearrange("p (b z) -> p b z", z=ZB)
        nc.vector.memset(qh3[:, :, 0:3], 0.0)
        nc.vector.memset(qh[:, NB * ZB:], 0.0)

        # ---- front end: d' -> tanh -> t^2 -> k -> H-band -> qh ----
        NDT = 24  # d-psum tiles of [96, 1024] per image
        for c in range(NDT):
            pd = psum.tile([H, 1024], f32, tag="ps")
            for piece in range(2):
                p_abs = c * 2 + piece          # 512-col piece index, 0..47
                g = p_abs // 2                 # w-quad group
                half = p_abs % 2
                base = 32 * (g % 3)
                nc.tensor.matmul(
                    pd[:, piece * 512:(piece + 1) * 512],
                    xt5g[g][:, i * H:(i + 1) * H],
                    crhsq_sb[base:base + 5, half * 512:(half + 1) * 512],
                    start=True,
                    stop=True,
                )
            tt = chunks.tile([H, 1024], f32, tag="t")
            nc.scalar.activation(tt[:], pd[:], AF.Tanh)
            kk = chunks.tile([H, 1024], f32, tag="k")
            nc.vector.tensor_tensor(kk[:], tt[:], tt[:], op=OP.mult)
            nc.vector.tensor_scalar(kk[:], kk[:], -0.25, 0.25, op0=OP.mult, op1=OP.add)
            ph = psum.tile([H, 1024], f32, tag="ps")
            for piece in range(2):
                nc.tensor.matmul(
                    ph[:, piece * 512:(piece + 1) * 512],
                    band_sb[:],
                    kk[:, piece * 512:(piece + 1) * 512],
                    start=True,
                    stop=True,
                )
            dst = qh3[:, :, 3 + 4 * c:3 + 4 * c + 4].transpose([0, 2, 1])
            if c % 2 == 0:
                nc.scalar.copy(dst, ph[:].rearrange("p (w b) -> p w b", b=NB))
            else:
                nc.vector.tensor_copy(dst, ph[:].rearrange("p (w b) -> p w b", b=NB))

        # ---- W window: single in-place prefix scan over the padded row,
        # then q[:, w, b] = P[99b + w + 5] - P[99b + w] (pads absorb edges) ----
        nc.vector.tensor_tensor_scan(
            qh[:], qh[:], qh[:], 0.0, op0=OP.add, op1=OP.bypass
        )

        for wc in range(W // 4):
            w0 = 4 * wc
            qt = chunks2.tile([H, 4, NB], f32, tag="q")
            if w0 + 9 <= ZB:
                hi = qh3[:, :, w0 + 5:w0 + 9].transpose([0, 2, 1])
                lo = qh3[:, :, w0:w0 + 4].transpose([0, 2, 1])
                nc.vector.tensor_tensor(qt[:], hi, lo, op=OP.subtract)
            else:
                for wi in range(4):
                    nc.vector.tensor_tensor(
                        qt[:, wi, :],
                        qh[:, w0 + 5 + wi::ZB][:, 0:NB],
                        qh[:, w0 + wi::ZB][:, 0:NB],
                        op=OP.subtract,
                    )
            ltile = chunks.tile([H, 1024], f32, tag="L")
            for j in range(4):
                w = w0 + j
                rcol = rtile[:, i * W + w:i * W + w + 1]
                nc.scalar.activation(
                    ltile[:, j * 256:(j + 1) * 256],
                    qt[:, j, :],
                    AF.Ln,
                    bias=bias_ap(EPS),
                    scale=rcol,
                )
            etile = chunks2.tile([H, 1024], f32, tag="e")
            nc.vector.tensor_tensor(
                etile[:].rearrange("p (a b) -> p a b", b=NB), qt[:], 
                ltile[:].rearrange("p (a b) -> p a b", b=NB), op=OP.mult
            )
            nc.vector.tensor_reduce(
                QL[:, i * W + w0:i * W + w0 + 4],
                etile[:].rearrange("p (a b) -> p a b", b=NB),
                axis=mybir.AxisListType.X,
                op=OP.add,
            )

    # E = r * QL ; write out
    ent = sm.tile([H, NW], f32)
    nc.vector.tensor_tensor(ent[:], rtile[:], QL[:], op=OP.mult)
    nc.vector.tensor_scalar(ent[:], ent[:], -1.0, None, op0=OP.mult)
    for i in range(NIMG):
        nc.sync.dma_start(ent_d[i], ent[:, i * W:(i + 1) * W])


def _get_compiled():
    if "nc" in _CACHE:
        return _CACHE["nc"]
    from contextlib import ExitStack

    import concourse.bass as bass
    import concourse.tile as tile
    from concourse import bacc, mybir

    f32 = mybir.dt.float32
    nc = bacc.Bacc("TRN2", target_bir_lowering=False, debug=False)
    x_d = nc.dram_tensor("x_sh", [NIMG, H, W], f32, kind="ExternalInput").ap()
    xt_d = nc.dram_tensor("xt_sh", [NIMG, W, H], f32, kind="ExternalInput").ap()
    crhs_d = nc.dram_tensor("crhs", [5, 4 * NB], f32, kind="ExternalInput").ap()
    band_d = nc.dram_tensor("bandh", [H, H], f32, kind="ExternalInput").ap()
    eye_d = nc.dram_tensor("i96", [H, H], f32, kind="ExternalInput").ap()
    ent_d = nc.dram_tensor("ent", [NIMG, H, W], f32, kind="ExternalOutput").ap()

    with tile.TileContext(nc) as tc:
        with ExitStack() as ctx:
            _emit_kernel(nc, tc, ctx, (x_d, xt_d, crhs_d, band_d, eye_d), (ent_d,))
    nc.compile()
    _CACHE["nc"] = nc
    return nc


def make_in_maps(x):
    """x: full [8, 3, 96, 96] -> list of 8 per-core input dicts."""
    x = np.ascontiguousarray(np.asarray(x, dtype=np.float32))
    imgs = x.reshape(NCORES * NIMG, H, W)
    crhs, band, eye = _build_consts()
    in_maps = []
    for c in range(NCORES):
        sh = np.ascontiguousarray(imgs[c * NIMG:(c + 1) * NIMG])
        in_maps.append(
            {
                "x_sh": sh,
                "xt_sh": np.ascontiguousarray(sh.transpose(0, 2, 1)),
                "crhs": crhs,
                "bandh": band,
                "i96": eye,
            }
        )
    return in_maps


def kernel(x):
    """Full inputs in, full outputs out. x: [8, 3, 96, 96] f32."""
    from concourse.bass_utils import run_bass_kernel_spmd

    nc = _get_compiled()
    in_maps = make_in_maps(x)
    res = run_bass_kernel_spmd(nc, in_maps, list(range(NCORES)))
    out = np.stack([res.results[c]["ent"] for c in range(NCORES)])  # [8, 3, H, W]
    return out.reshape(8, 3, H, W).astype(np.float32)


# revision 23
# speedup vs baseline: 1.1797x; 1.0022x over previous
"""Trainium2 Bass kernel for nn_Entropy (histogram_binning): per-pixel Shannon
entropy of a 5x5-window KDE histogram over 256 intensity bins.

Math (validated in f32 vs reference):
  k(x,b) = sigmoid'(10(x-b)) = 0.25*(1 - tanh^2(5x-5b))   [exact identity]
  q[h,w,b] = 5x5 window sum of k;  S = sum_b q;  p = q/(S+EPS)
  out = -sum_b p*ln(p+EPS)
     computed as E = -r * sum_b q*ln(r*q+EPS),  r = 1/(S+EPS)
  S is computed analytically per pixel: s(x) = sum_o in {-1,0,1,2} of
  0.25*(1-tanh^2(5*frac(x)-5o)) masked at intensity range edges, then
  5x5-window-summed (tiny [96,96] work instead of a 256-bin reduction).

Layout per image: partitions = h (96), free = (w, b) with b inner (24576).
  - d' = 5x - 5b built by TensorE: rank-97 matmul with a shipped constant
    moving tensor CRHS[w', w*256+b] = 5*delta(w'==w), row 96 = -5b; the
    stationary is [5*x^T ; ones].
  - tanh on ScalarE evacuating PSUM; k = 0.25 - 0.25*t^2 on VectorE.
  - H-window: banded-matrix matmul (TensorE).  W-window: 5 shifted
    identity matmuls accumulating in PSUM (TensorE).
  - backend per w: L = ln(r*q + EPS) on ScalarE (per-partition scale AP),
    QL = sum_b -(q*L) via fused tensor_tensor_reduce on VectorE; E = r*QL.

Sharding: B*C = 24 images split 3-per-core across 8 cores; no collectives.
Self-contained: hardcodes shapes; builds/compiles the Bass module on first
call and reuses it.
"""

import sys

sys.path.insert(0, "/opt/trn_rl_repo")

import numpy as np

H = 96
W = 96
NB = 256
NIMG = 3          # images per core
NCORES = 8
FREE = W * NB     # 24576
EPS = 1e-10

_CACHE = {}


def _build_consts():
    # CRHSQ [5, 1024]: rows j=0..3 select w-offset j (value 1.0 over that
    # b-block); row 4 = -5*b tiled (the stationary carries 5*x^T and ones)
    crhs = np.zeros((5, 4 * NB), dtype=np.float32)
    for j in range(4):
        crhs[j, j * NB:(j + 1) * NB] = 1.0
    b = np.arange(NB, dtype=np.float32)
    crhs[4, :] = np.tile(-5.0 * b, 4)
    # banded H-window matrix [96, 96] (symmetric)
    hh = np.arange(H)
    band = (np.abs(hh[:, None] - hh[None, :]) <= 2).astype(np.float32)
    eye = np.eye(H, dtype=np.float32)
    return crhs, band, eye


def _emit_kernel(nc, tc, ctx, ins, outs):
    import concourse.bass as bass
    from concourse import mybir

    f32 = mybir.dt.float32
    AF = mybir.ActivationFunctionType
    OP = mybir.AluOpType

    x_d, xt_d, crhs_d, band_d, eye_d = ins
    (ent_d,) = outs

    consts = ctx.enter_context(tc.tile_pool(name="consts", bufs=1))
    big = ctx.enter_context(tc.tile_pool(name="big", bufs=1))
    sm = ctx.enter_context(tc.tile_pool(name="sm", bufs=1))
    chunks = ctx.enter_context(tc.tile_pool(name="chunks", bufs=3))
    chunks2 = ctx.enter_context(tc.tile_pool(name="chunks2", bufs=2))
    psum = ctx.enter_context(tc.tile_pool(name="psum", bufs=4, space="PSUM"))

    # ---- load constants / inputs ----
    crhsq_sb = consts.tile([69, 4 * NB], f32)
    for k3 in range(3):
        nc.sync.dma_start(crhsq_sb[32 * k3:32 * k3 + 5, :], crhs_d[:])
    band_sb = consts.tile([H, H], f32)
    nc.sync.dma_start(band_sb[:], band_d[:])

    xall = consts.tile([H, NIMG * W], f32)      # x[h, (i,w)]
    xtall = consts.tile([W, NIMG * H], f32)     # xT[w, (i,h)]
    for i in range(NIMG):
        nc.sync.dma_start(xall[:, i * W:(i + 1) * W], x_d[i])
        nc.sync.dma_start(xtall[:, i * H:(i + 1) * H], xt_d[i])

    # 24 stationary groups of 5 rows = [5*xT rows 4g..4g+3 ; ones], each in
    # its own tile (matmul lhsT base partition must be 0); the ones row is
    # DMA'd (compute engines can't write at partition offset 4)
    ones_sb = consts.tile([1, NIMG * H], f32)
    nc.vector.memset(ones_sb[:], 1.0)
    xt5_all = consts.tile([W, NIMG * H], f32)
    nc.vector.tensor_scalar(xt5_all[:], xtall[:], 5.0, None, op0=OP.mult)
    # 3 groups per tile at base partitions 0/32/64 (matmul lhsT constraint)
    xt5g = []
    for tg in range(8):
        gt = consts.tile([69, NIMG * H], f32, tag=f"xt5g{tg}")
        for k3 in range(3):
            g = tg * 3 + k3
            base = 32 * k3
            nc.sync.dma_start(gt[base:base + 4, :], xt5_all[4 * g:4 * g + 4, :])
            nc.sync.dma_start(gt[base + 4:base + 5, :], ones_sb[:])
            xt5g.append(gt[base:base + 5])

    # bias constant tiles (activation float bias needs a registered AP)
    bias_tiles = {}

    def bias_ap(val):
        if val not in bias_tiles:
            t = consts.tile([H, 1], f32, tag=f"bias{val}")
            nc.vector.memset(t[:], val)
            bias_tiles[val] = t
        return bias_tiles[val][:]

    # =====================  S path (tiny, [96, 288])  =====================
    NW = NIMG * W
    i32 = mybir.dt.int32
    ni = sm.tile([H, NW], i32)
    nc.vector.tensor_copy(ni[:], xall[:])     # f32 -> i32 (trunc or round)
    nf = sm.tile([H, NW], f32)
    nc.vector.tensor_copy(nf[:], ni[:])       # back to f32
    u = sm.tile([H, NW], f32)
    nc.vector.tensor_tensor(u[:], xall[:], nf[:], op=OP.subtract)
    # taps o in {-2..2}: v_o = tanh(5u - 5o); sq_o = v_o^2; masks from n:
    # bin n+o valid iff 0 <= n+o <= 255 (correct for either cast semantics)
    taps = (-2, -1, 0, 1, 2)
    sq = {}
    for o in taps:
        v = sm.tile([H, NW], f32, tag=f"v{o}")
        nc.scalar.activation(v[:], u[:], AF.Tanh, bias=bias_ap(-5.0 * o), scale=5.0)
        s2 = sm.tile([H, NW], f32, tag=f"sq{o}")
        nc.scalar.activation(s2[:], v[:], AF.Square)
        sq[o] = s2
    masks = {}
    for o in taps:
        if o == 0:
            continue
        m = sm.tile([H, NW], f32, tag=f"m{o}")
        if o < 0:
            nc.vector.tensor_scalar(m[:], nf[:], float(-o), None, op0=OP.is_ge)
        else:
            nc.vector.tensor_scalar(m[:], nf[:], float(255 - o), None, op0=OP.is_le)
        masks[o] = m
    # cnt = 1 + sum_o masks ; ssum = sq0 + sum_o m_o*sq_o
    cnt = sm.tile([H, NW], f32)
    nc.vector.tensor_tensor(cnt[:], masks[-2][:], masks[-1][:], op=OP.add)
    nc.vector.tensor_tensor(cnt[:], cnt[:], masks[1][:], op=OP.add)
    nc.vector.tensor_tensor(cnt[:], cnt[:], masks[2][:], op=OP.add)
    nc.vector.tensor_scalar(cnt[:], cnt[:], 1.0, None, op0=OP.add)
    ssum = sm.tile([H, NW], f32)
    nc.vector.tensor_copy(ssum[:], sq[0][:])
    for o in (-2, -1, 1, 2):
        t_m = sm.tile([H, NW], f32, tag=f"tm{o}")
        nc.vector.tensor_tensor(t_m[:], masks[o][:], sq[o][:], op=OP.mult)
        nc.vector.tensor_tensor(ssum[:], ssum[:], t_m[:], op=OP.add)
    # s_pix = 0.25*cnt - 0.25*ssum
    spix = sm.tile([H, NW], f32)
    nc.vector.tensor_tensor(spix[:], cnt[:], ssum[:], op=OP.subtract)
    nc.vector.tensor_scalar(spix[:], spix[:], 0.25, None, op0=OP.mult)
    # H window via band matmul
    ps_s = psum.tile([H, 1024], f32, tag="ps")
    nc.tensor.matmul(ps_s[:, 0:NW], band_sb[:], spix[:], start=True, stop=True)
    sh = sm.tile([H, NW], f32)
    nc.scalar.copy(sh[:], ps_s[:, 0:NW])
    # W window via padded shifted adds: shp [H, NIMG, 100] zero-padded
    shp = sm.tile([H, NIMG, W + 4], f32)
    nc.vector.memset(shp[:], 0.0)
    for i in range(NIMG):
        nc.vector.tensor_copy(shp[:, i, 2:2 + W], sh[:, i * W:(i + 1) * W])
    swin = sm.tile([H, NIMG, W], f32)
    nc.vector.tensor_tensor(swin[:], shp[:, :, 0:W], shp[:, :, 1:1 + W], op=OP.add)
    for j in (2, 3, 4):
        nc.vector.tensor_tensor(swin[:], swin[:], shp[:, :, j:j + W], op=OP.add)
    # r = 1/(S + EPS)
    rtile = sm.tile([H, NW], f32)
    sw_flat = swin[:].rearrange("p a b -> p (a b)")
    nc.vector.tensor_scalar(rtile[:], sw_flat, EPS, None, op0=OP.add)
    nc.vector.reciprocal(rtile[:], rtile[:])

    # =====================  main dense path, software-pipelined  ============
    QL = sm.tile([H, NW], f32)

    ZB = 99  # per-bin block: 3 zero pads + 96 w columns
    qh_views = {}

    def emit_front(i):
        qh = big.tile([H, NB * ZB + 8], f32, tag="qh")
        qh3 = qh[:, 0:NB * ZB].rearrange("p (b z) -> p b z", z=ZB)
        nc.vector.memset(qh3[:, :, 0:3], 0.0)
        nc.vector.memset(qh[:, NB * ZB:], 0.0)

        # ---- front end: d' -> tanh -> t^2 -> k -> H-band -> qh ----
        NDT = 24  # d-psum tiles of [96, 1024] per image
        for c in range(NDT):
            pd = psum.tile([H, 1024], f32, tag="ps")
            for piece in range(2):
                p_abs = c * 2 + piece          # 512-col piece index, 0..47
                g = p_abs // 2                 # w-quad group
                half = p_abs % 2
                base = 32 * (g % 3)
                nc.tensor.matmul(
                    pd[:, piece * 512:(piece + 1) * 512],
                    xt5g[g][:, i * H:(i + 1) * H],
                    crhsq_sb[base:base + 5, half * 512:(half + 1) * 512],
                    start=True,
                    stop=True,
                )
            tt = chunks.tile([H, 1024], f32, tag="t")
            nc.scalar.activation(tt[:], pd[:], AF.Tanh)
            kk = chunks.tile([H, 1024], f32, tag="k")
            nc.vector.tensor_tensor(kk[:], tt[:], tt[:], op=OP.mult)
            nc.vector.tensor_scalar(kk[:], kk[:], -0.25, 0.25, op0=OP.mult, op1=OP.add)
            ph = psum.tile([H, 1024], f32, tag="ps")
            for piece in range(2):
                nc.tensor.matmul(
                    ph[:, piece * 512:(piece + 1) * 512],
                    band_sb[:],
                    kk[:, piece * 512:(piece + 1) * 512],
                    start=True,
                    stop=True,
                )
            dst = qh3[:, :, 3 + 4 * c:3 + 4 * c + 4].transpose([0, 2, 1])
            if c % 2 == 0:
                nc.scalar.copy(dst, ph[:].rearrange("p (w b) -> p w b", b=NB))
            else:
                nc.vector.tensor_copy(dst, ph[:].rearrange("p (w b) -> p w b", b=NB))

        # ---- W window: single in-place prefix scan over the padded row,
        # then q[:, w, b] = P[99b + w + 5] - P[99b + w] (pads absorb edges) ----
        nc.vector.tensor_tensor_scan(
            qh[:], qh[:], qh[:], 0.0, op0=OP.add, op1=OP.bypass
        )
        qh_views[i] = (qh, qh3)

    def emit_backend(i):
        qh, qh3 = qh_views.pop(i)
        for wc in range(W // 4):
            w0 = 4 * wc
            qt = chunks2.tile([H, 4, NB], f32, tag="q")
            if w0 + 9 <= ZB:
                hi = qh3[:, :, w0 + 5:w0 + 9].transpose([0, 2, 1])
                lo = qh3[:, :, w0:w0 + 4].transpose([0, 2, 1])
                nc.vector.tensor_tensor(qt[:], hi, lo, op=OP.subtract)
            else:
                for wi in range(4):
                    nc.vector.tensor_tensor(
                        qt[:, wi, :],
                        qh[:, w0 + 5 + wi::ZB][:, 0:NB],
                        qh[:, w0 + wi::ZB][:, 0:NB],
                        op=OP.subtract,
                    )
            ltile = chunks.tile([H, 1024], f32, tag="L")
            for j in range(4):
                w = w0 + j
                rcol = rtile[:, i * W + w:i * W + w + 1]
                nc.scalar.activation(
                    ltile[:, j * 256:(j + 1) * 256],
                    qt[:, j, :],
                    AF.Ln,
                    bias=bias_ap(EPS),
                    scale=rcol,
                )
            etile = chunks2.tile([H, 1024], f32, tag="e")
            nc.vector.tensor_tensor(
                etile[:].rearrange("p (a b) -> p a b", b=NB), qt[:], 
                ltile[:].rearrange("p (a b) -> p a b", b=NB), op=OP.mult
            )
            nc.vector.tensor_reduce(
                QL[:, i * W + w0:i * W + w0 + 4],
                etile[:].rearrange("p (a b) -> p a b", b=NB),
                axis=mybir.AxisListType.X,
                op=OP.add,
            )

    for i in range(NIMG):
        emit_front(i)
        if i > 0:
            emit_backend(i - 1)
    emit_backend(NIMG - 1)

    # E = r * QL ; write out
    ent = sm.tile([H, NW], f32)
    nc.vector.tensor_tensor(ent[:], rtile[:], QL[:], op=OP.mult)
    nc.vector.tensor_scalar(ent[:], ent[:], -1.0, None, op0=OP.mult)
    for i in range(NIMG):
        nc.sync.dma_start(ent_d[i], ent[:, i * W:(i + 1) * W])


def _get_compiled():
    if "nc" in _CACHE:
        return _CACHE["nc"]
    from contextlib import ExitStack

    import concourse.bass as bass
    import concourse.tile as tile
    from concourse import bacc, mybir

    f32 = mybir.dt.float32
    nc = bacc.Bacc("TRN2", target_bir_lowering=False, debug=False)
    x_d = nc.dram_tensor("x_sh", [NIMG, H, W], f32, kind="ExternalInput").ap()
    xt_d = nc.dram_tensor("xt_sh", [NIMG, W, H], f32, kind="ExternalInput").ap()
    crhs_d = nc.dram_tensor("crhs", [5, 4 * NB], f32, kind="ExternalInput").ap()
    band_d = nc.dram_tensor("bandh", [H, H], f32, kind="ExternalInput").ap()
    eye_d = nc.dram_tensor("i96", [H, H], f32, kind="ExternalInput").ap()
    ent_d = nc.dram_tensor("ent", [NIMG, H, W], f32, kind="ExternalOutput").ap()

    with tile.TileContext(nc) as tc:
        with ExitStack() as ctx:
            _emit_kernel(nc, tc, ctx, (x_d, xt_d, crhs_d, band_d, eye_d), (ent_d,))
    nc.compile()
    _CACHE["nc"] = nc
    return nc


def make_in_maps(x):
    """x: full [8, 3, 96, 96] -> list of 8 per-core input dicts."""
    x = np.ascontiguousarray(np.asarray(x, dtype=np.float32))
    imgs = x.reshape(NCORES * NIMG, H, W)
    crhs, band, eye = _build_consts()
    in_maps = []
    for c in range(NCORES):
        sh = np.ascontiguousarray(imgs[c * NIMG:(c + 1) * NIMG])
        in_maps.append(
            {
                "x_sh": sh,
                "xt_sh": np.ascontiguousarray(sh.transpose(0, 2, 1)),
                "crhs": crhs,
                "bandh": band,
                "i96": eye,
            }
        )
    return in_maps


def kernel(x):
    """Full inputs in, full outputs out. x: [8, 3, 96, 96] f32."""
    from concourse.bass_utils import run_bass_kernel_spmd

    nc = _get_compiled()
    in_maps = make_in_maps(x)
    res = run_bass_kernel_spmd(nc, in_maps, list(range(NCORES)))
    out = np.stack([res.results[c]["ent"] for c in range(NCORES)])  # [8, 3, H, W]
    return out.reshape(8, 3, H, W).astype(np.float32)


# revision 25
# speedup vs baseline: 1.4329x; 1.2147x over previous
"""Trainium2 Bass kernel for nn_Entropy (histogram_binning): per-pixel Shannon
entropy of a 5x5-window KDE histogram over 256 intensity bins.

Math (validated in f32 vs reference):
  k(x,b) = sigmoid'(10(x-b)) = 0.25*(1 - tanh^2(5x-5b))   [exact identity]
  q[h,w,b] = 5x5 window sum of k;  S = sum_b q;  p = q/(S+EPS)
  out = -sum_b p*ln(p+EPS) = -r * sum_b q*ln(r*q+EPS),  r = 1/(S+EPS)
  S comes analytically per pixel from 5 taps of the KDE kernel around
  frac(x) (range-masked), then a tiny 5x5 window sum.

Layout per (image, bin-half) stripe: partitions = h (96), free = (w, b).
  - d' = 5x - 5b on TensorE: K=9 matmuls; stationary = [5*x^T(8 w-rows);
    ones], moving = tiny shipped selector constant.
  - tanh on ScalarE (evacuates PSUM); k = 0.25 - 0.25 t^2 on VectorE.
  - H-window: banded-matrix matmul (TensorE) -> PSUM, evacuated by
    ScalarE into a w-inner padded stripe [b-block: 3 zero pads + 96 w].
  - W-window: one in-place VectorE prefix scan per stripe over the padded
    row; q[w,b] = P[99b+w+5] - P[99b+w] (pads absorb all edges).
  - backend per w: L = ln(r*q + EPS) on ScalarE (per-partition scale AP),
    e = q*L and QL = sum_b(e) on VectorE; E = -r*QL.

Stripes are software-pipelined (3 stripe buffers) so image i+1's front end
overlaps image i's backend. Sharding: B*C = 24 images, 3 per core across 8
cores; no collectives. Self-contained; compiled once per process.
"""

import sys

sys.path.insert(0, "/opt/trn_rl_repo")

import numpy as np

H = 96
W = 96
NB = 256
NBH = 128         # bins per stripe (half)
NIMG = 3
NCORES = 8
EPS = 1e-10
ZB = 99           # per-bin block in a stripe: 3 zero pads + 96 w cols
WQ = 8            # w rows per stationary group
NG = W // WQ      # 12 groups

_CACHE = {}


def _build_consts():
    # selector constants per bin-half: [9, WQ*NBH]; rows j=0..7 mark w-offset
    # j over that bin-block; row 8 = -5*b
    crhs = []
    for half in range(2):
        c = np.zeros((9, WQ * NBH), dtype=np.float32)
        for j in range(WQ):
            c[j, j * NBH:(j + 1) * NBH] = 1.0
        b = np.arange(NBH, dtype=np.float32) + half * NBH
        c[8, :] = np.tile(-5.0 * b, WQ)
        crhs.append(c)
    hh = np.arange(H)
    band = (np.abs(hh[:, None] - hh[None, :]) <= 2).astype(np.float32)
    return crhs[0], crhs[1], band


def _emit_kernel(nc, tc, ctx, ins, outs):
    from concourse import mybir

    f32 = mybir.dt.float32
    i32 = mybir.dt.int32
    AF = mybir.ActivationFunctionType
    OP = mybir.AluOpType

    x_d, xt_d, crhs0_d, crhs1_d, band_d = ins
    (ent_d,) = outs
    NW = NIMG * W

    consts = ctx.enter_context(tc.tile_pool(name="consts", bufs=1))
    stripes = ctx.enter_context(tc.tile_pool(name="stripes", bufs=2))
    sm = ctx.enter_context(tc.tile_pool(name="sm", bufs=1))
    chunks = ctx.enter_context(tc.tile_pool(name="chunks", bufs=2))
    psum = ctx.enter_context(tc.tile_pool(name="psum", bufs=4, space="PSUM"))

    # ---- constants / inputs ----
    crhs_sb = []
    for half, cd in ((0, crhs0_d), (1, crhs1_d)):
        t = consts.tile([73, WQ * NBH], f32, tag=f"crhs{half}")
        for k3 in range(3):
            nc.sync.dma_start(t[32 * k3:32 * k3 + 9, :], cd[:])
        crhs_sb.append(t)
    band_sb = consts.tile([H, H], f32)
    nc.sync.dma_start(band_sb[:], band_d[:])

    xall = consts.tile([H, NW], f32)
    xtall = consts.tile([W, NIMG * H], f32)
    for i in range(NIMG):
        nc.sync.dma_start(xall[:, i * W:(i + 1) * W], x_d[i])
        nc.sync.dma_start(xtall[:, i * H:(i + 1) * H], xt_d[i])

    ones_sb = consts.tile([1, NIMG * H], f32)
    nc.vector.memset(ones_sb[:], 1.0)
    xt5_all = consts.tile([W, NIMG * H], f32)
    nc.vector.tensor_scalar(xt5_all[:], xtall[:], 5.0, None, op0=OP.mult)
    # stationary groups [9 rows: 5*xT(8 w) ; ones], 3 per tile at bases 0/32/64
    xt9g = []
    for tg in range(4):
        gt = consts.tile([73, NIMG * H], f32, tag=f"xt9g{tg}")
        for k3 in range(3):
            g = tg * 3 + k3
            base = 32 * k3
            nc.sync.dma_start(gt[base:base + 8, :], xt5_all[8 * g:8 * g + 8, :])
            nc.sync.dma_start(gt[base + 8:base + 9, :], ones_sb[:])
            xt9g.append(gt[base:base + 9])

    bias_tiles = {}

    def bias_ap(val):
        if val not in bias_tiles:
            t = consts.tile([H, 1], f32, tag=f"bias{val}")
            nc.vector.memset(t[:], val)
            bias_tiles[val] = t
        return bias_tiles[val][:]

    # =====================  S path (tiny, [96, 288])  =====================
    ni = sm.tile([H, NW], i32)
    nc.vector.tensor_copy(ni[:], xall[:])
    nf = sm.tile([H, NW], f32)
    nc.vector.tensor_copy(nf[:], ni[:])
    u = sm.tile([H, NW], f32)
    nc.vector.tensor_tensor(u[:], xall[:], nf[:], op=OP.subtract)
    taps = (-2, -1, 0, 1, 2)
    sq = {}
    for o in taps:
        v = sm.tile([H, NW], f32, tag=f"v{o}")
        nc.scalar.activation(v[:], u[:], AF.Tanh, bias=bias_ap(-5.0 * o), scale=5.0)
        s2 = sm.tile([H, NW], f32, tag=f"sq{o}")
        nc.scalar.activation(s2[:], v[:], AF.Square)
        sq[o] = s2
    masks = {}
    for o in taps:
        if o == 0:
            continue
        m = sm.tile([H, NW], f32, tag=f"m{o}")
        if o < 0:
            nc.vector.tensor_scalar(m[:], nf[:], float(-o), None, op0=OP.is_ge)
        else:
            nc.vector.tensor_scalar(m[:], nf[:], float(255 - o), None, op0=OP.is_le)
        masks[o] = m
    cnt = sm.tile([H, NW], f32)
    nc.vector.tensor_tensor(cnt[:], masks[-2][:], masks[-1][:], op=OP.add)
    nc.vector.tensor_tensor(cnt[:], cnt[:], masks[1][:], op=OP.add)
    nc.vector.tensor_tensor(cnt[:], cnt[:], masks[2][:], op=OP.add)
    nc.vector.tensor_scalar(cnt[:], cnt[:], 1.0, None, op0=OP.add)
    ssum = sm.tile([H, NW], f32)
    nc.vector.tensor_copy(ssum[:], sq[0][:])
    for o in (-2, -1, 1, 2):
        t_m = sm.tile([H, NW], f32, tag=f"tm{o}")
        nc.vector.tensor_tensor(t_m[:], masks[o][:], sq[o][:], op=OP.mult)
        nc.vector.tensor_tensor(ssum[:], ssum[:], t_m[:], op=OP.add)
    spix = sm.tile([H, NW], f32)
    nc.vector.tensor_tensor(spix[:], cnt[:], ssum[:], op=OP.subtract)
    nc.vector.tensor_scalar(spix[:], spix[:], 0.25, None, op0=OP.mult)
    ps_s = psum.tile([H, 1024], f32, tag="ps")
    nc.tensor.matmul(ps_s[:, 0:NW], band_sb[:], spix[:], start=True, stop=True)
    sh = sm.tile([H, NW], f32)
    nc.scalar.copy(sh[:], ps_s[:, 0:NW])
    shp = sm.tile([H, NIMG, W + 4], f32)
    nc.vector.memset(shp[:], 0.0)
    for i in range(NIMG):
        nc.vector.tensor_copy(shp[:, i, 2:2 + W], sh[:, i * W:(i + 1) * W])
    swin = sm.tile([H, NIMG, W], f32)
    nc.vector.tensor_tensor(swin[:], shp[:, :, 0:W], shp[:, :, 1:1 + W], op=OP.add)
    for j in (2, 3, 4):
        nc.vector.tensor_tensor(swin[:], swin[:], shp[:, :, j:j + W], op=OP.add)
    rtile = sm.tile([H, NW], f32)
    sw_flat = swin[:].rearrange("p a b -> p (a b)")
    nc.vector.tensor_scalar(rtile[:], sw_flat, EPS, None, op0=OP.add)
    nc.vector.reciprocal(rtile[:], rtile[:])

    # =====================  main path: per (image, bin-half) stripe  ========
    QL = sm.tile([H, NW], f32)
    stripe_store = {}

    def emit_front(i, half):
        qh = stripes.tile([H, NBH * ZB + 8], f32, tag="qh")
        qh3 = qh[:, 0:NBH * ZB].rearrange("p (b z) -> p b z", z=ZB)
        nc.vector.memset(qh3[:, :, 0:3], 0.0)
        nc.vector.memset(qh[:, NBH * ZB:], 0.0)

        for c in range(NG // 2):  # chunks of 2 w-groups = [96, 2048] cols
            pd = psum.tile([H, 1024], f32, tag="ps")
            pd2 = psum.tile([H, 1024], f32, tag="ps")
            for piece, pt in ((0, pd), (1, pd2)):
                g = 2 * c + piece
                base = 32 * (g % 3)
                nc.tensor.matmul(
                    pt[:, 0:512],
                    xt9g[g][:, i * H:(i + 1) * H],
                    crhs_sb[half][base:base + 9, 0:512],
                    start=True, stop=True,
                )
                nc.tensor.matmul(
                    pt[:, 512:1024],
                    xt9g[g][:, i * H:(i + 1) * H],
                    crhs_sb[half][base:base + 9, 512:1024],
                    start=True, stop=True,
                )
            tt = chunks.tile([H, 2048], f32, tag="t")
            nc.scalar.activation(tt[:, 0:1024], pd[:], AF.Tanh)
            nc.scalar.activation(tt[:, 1024:2048], pd2[:], AF.Tanh)
            kk = chunks.tile([H, 2048], f32, tag="k")
            nc.vector.tensor_tensor(kk[:], tt[:], tt[:], op=OP.mult)
            nc.vector.tensor_scalar(kk[:], kk[:], -0.25, 0.25, op0=OP.mult, op1=OP.add)
            for piece in range(2):
                ph = psum.tile([H, 1024], f32, tag="ps")
                for pp in range(2):
                    nc.tensor.matmul(
                        ph[:, pp * 512:(pp + 1) * 512],
                        band_sb[:],
                        kk[:, piece * 1024 + pp * 512:piece * 1024 + (pp + 1) * 512],
                        start=True, stop=True,
                    )
                # evac: chunk piece covers w-group g = 2c+piece (8 w), all bins
                g = 2 * c + piece
                dst = qh3[:, :, 3 + 8 * g:3 + 8 * g + 8].transpose([0, 2, 1])
                nc.scalar.copy(dst, ph[:].rearrange("p (w b) -> p w b", b=NBH))

        nc.vector.tensor_tensor_scan(
            qh[:], qh[:], qh[:], 0.0, op0=OP.add, op1=OP.bypass
        )
        stripe_store[(i, half)] = (qh, qh3)

    def emit_backend(i):
        qhs = [stripe_store.pop((i, 0)), stripe_store.pop((i, 1))]
        for wc in range(W // 4):
            w0 = 4 * wc
            qt = chunks.tile([H, 4, NB], f32, tag="q")
            for half, (qh, qh3) in enumerate(qhs):
                if w0 + 9 <= ZB:
                    hi = qh3[:, :, w0 + 5:w0 + 9].transpose([0, 2, 1])
                    lo = qh3[:, :, w0:w0 + 4].transpose([0, 2, 1])
                    nc.vector.tensor_tensor(
                        qt[:, :, half * NBH:(half + 1) * NBH], hi, lo,
                        op=OP.subtract,
                    )
                else:
                    for wi in range(4):
                        nc.vector.tensor_tensor(
                            qt[:, wi, half * NBH:(half + 1) * NBH],
                            qh[:, w0 + 5 + wi::ZB][:, 0:NBH],
                            qh[:, w0 + wi::ZB][:, 0:NBH],
                            op=OP.subtract,
                        )
            ltile = chunks.tile([H, 1024], f32, tag="L")
            for j in range(4):
                w = w0 + j
                rcol = rtile[:, i * W + w:i * W + w + 1]
                nc.scalar.activation(
                    ltile[:, j * 256:(j + 1) * 256],
                    qt[:, j, :],
                    AF.Ln,
                    bias=bias_ap(EPS),
                    scale=rcol,
                )
            l3 = ltile[:].rearrange("p (a b) -> p a b", b=NB)
            nc.vector.tensor_tensor(l3, qt[:], l3, op=OP.mult)
            nc.vector.tensor_reduce(
                QL[:, i * W + w0:i * W + w0 + 4],
                l3,
                axis=mybir.AxisListType.X,
                op=OP.add,
            )

    emit_front(0, 0)
    emit_front(0, 1)
    emit_front(1, 0)
    emit_backend(0)
    emit_front(1, 1)
    emit_front(2, 0)
    emit_backend(1)
    emit_front(2, 1)
    emit_backend(2)

    # E = -(r * QL) ; write out
    ent = sm.tile([H, NW], f32)
    nc.vector.tensor_tensor(ent[:], rtile[:], QL[:], op=OP.mult)
    nc.vector.tensor_scalar(ent[:], ent[:], -1.0, None, op0=OP.mult)
    for i in range(NIMG):
        nc.sync.dma_start(ent_d[i], ent[:, i * W:(i + 1) * W])


def _get_compiled():
    if "nc" in _CACHE:
        return _CACHE["nc"]
    from contextlib import ExitStack

    import concourse.tile as tile
    from concourse import bacc, mybir

    f32 = mybir.dt.float32
    nc = bacc.Bacc("TRN2", target_bir_lowering=False, debug=False)
    x_d = nc.dram_tensor("x_sh", [NIMG, H, W], f32, kind="ExternalInput").ap()
    xt_d = nc.dram_tensor("xt_sh", [NIMG, W, H], f32, kind="ExternalInput").ap()
    crhs0_d = nc.dram_tensor("crhs0", [9, WQ * NBH], f32, kind="ExternalInput").ap()
    crhs1_d = nc.dram_tensor("crhs1", [9, WQ * NBH], f32, kind="ExternalInput").ap()
    band_d = nc.dram_tensor("bandh", [H, H], f32, kind="ExternalInput").ap()
    ent_d = nc.dram_tensor("ent", [NIMG, H, W], f32, kind="ExternalOutput").ap()

    with tile.TileContext(nc) as tc:
        with ExitStack() as ctx:
            _emit_kernel(
                nc, tc, ctx, (x_d, xt_d, crhs0_d, crhs1_d, band_d), (ent_d,)
            )
    nc.compile()
    _CACHE["nc"] = nc
    return nc


def make_in_maps(x):
    """x: full [8, 3, 96, 96] -> list of 8 per-core input dicts."""
    x = np.ascontiguousarray(np.asarray(x, dtype=np.float32))
    imgs = x.reshape(NCORES * NIMG, H, W)
    crhs0, crhs1, band = _build_consts()
    in_maps = []
    for c in range(NCORES):
        sh = np.ascontiguousarray(imgs[c * NIMG:(c + 1) * NIMG])
        in_maps.append(
            {
                "x_sh": sh,
                "xt_sh": np.ascontiguousarray(sh.transpose(0, 2, 1)),
                "crhs0": crhs0,
                "crhs1": crhs1,
                "bandh": band,
            }
        )
    return in_maps


def kernel(x):
    """Full inputs in, full outputs out. x: [8, 3, 96, 96] f32."""
    from concourse.bass_utils import run_bass_kernel_spmd

    nc = _get_compiled()
    in_maps = make_in_maps(x)
    res = run_bass_kernel_spmd(nc, in_maps, list(range(NCORES)))
    out = np.stack([res.results[c]["ent"] for c in range(NCORES)])
    return out.reshape(8, 3, H, W).astype(np.float32)
